# revision 27
# baseline (speedup 1.0000x reference)
"""Trainium2 Bass kernel for a 6-layer GPT-style transformer (ALiBi + causal),
data-parallel over batch across 8 NeuronCores (1 sequence per core).

v2 design (vs baseline):
  - all matmul operands bf16 (fp32 PSUM accumulation), FWL weight loads,
    half DMA traffic
  - ALiBi folded into per-partition exp bias + fp32 rank-1 "anchor
    injection" matmuls (no per-(kb,qb) rank-1 ALiBi matmuls)
  - causal windowing by slope: heads 8,0 -> 128-query blocks w/ 2 key
    strips; heads 9,1,10,2,11,3 -> 256-query blocks w/ 3 strips; heads
    4,6,5,7 -> full causal
  - head pairs interleaved for PE row/col-tile concurrency: scores use
    row groups (kpair partitions 0:64 / 64:128), o/r use col groups
    (po partitions 0:64 / 64:128), shared softmax-normalize ops
  - causal mask applied post-exp as bf16 triangle multiply on DVE
  - paired out-projection: contraction 128-deep over head pairs
"""

import math
import os
import sys

import numpy as np

sys.path.insert(0, "/opt/trn_rl_repo")

import concourse.bass as bass  # noqa: E402
import concourse.mybir as mybir  # noqa: E402
import concourse.tile as tile  # noqa: E402
from concourse import bacc  # noqa: E402

P = 128
B, N, E, H, DEPTH, A = 8, 1024, 768, 12, 6, 7
DH = E // H  # 64
F = 4 * E  # 3072
NT = N // P  # 8 token tiles
EC = E // P  # 6
FC = F // P  # 24
SCALE = DH ** -0.5  # 0.125
EPS = 1e-6

# pair g: (even member -> partitions 0:64, odd member -> 64:128)
PAIRS = [(8, 0), (9, 1), (10, 2), (11, 3), (4, 6), (5, 7)]
PERM = [h for pr in PAIRS for h in pr]
KIND = ["g128", "g256", "g256", "g256", "full", "full"]

f32 = mybir.dt.float32
f32r = mybir.dt.float32r
bf16 = mybir.dt.bfloat16
AF = mybir.ActivationFunctionType
ALU = mybir.AluOpType


def _slopes(n):
    def p2(n):
        start = 2 ** (-(2 ** -(math.log2(n) - 3)))
        return [start * start**i for i in range(n)]

    if math.log2(n).is_integer():
        return p2(n)
    c = 2 ** math.floor(math.log2(n))
    return p2(c) + _slopes(2 * c)[0::2][: n - c]


def build_program(finalize=True, depth=DEPTH):
    nc = bacc.Bacc()

    dp = nc.declare_dram_parameter
    x_d = dp("x", [N, E], f32, isOutput=False)
    wqkv_d = dp("wqkv_p", [DEPTH, E, 3 * E], bf16, isOutput=False)
    wo_d = dp("wo_p", [DEPTH, P, EC, E], bf16, isOutput=False)
    w1_d = dp("w1_p", [DEPTH, E, F], bf16, isOutput=False)
    b1_d = dp("b1_t", [DEPTH, P, FC], f32, isOutput=False)
    w2_d = dp("w2_p", [DEPTH, F, E], bf16, isOutput=False)
    hw1_d = dp("hw1_p", [E, E], bf16, isOutput=False)
    hb1_d = dp("hb1_t", [P, EC], f32, isOutput=False)
    hw2_d = dp("hw2_p", [E, 8], bf16, isOutput=False)
    ab2_d = dp("ab2", [P, H], f32, isOutput=False)
    abf_d = dp("abf", [P, 4, NT], f32, isOutput=False)
    inj_d = dp("inj", [1, 8, 768], f32, isOutput=False)
    id_d = dp("ident_bf", [P, P], bf16, isOutput=False)
    tri_d = dp("tri01", [P, P], bf16, isOutput=False)
    out_d = dp("out", [N, A], f32, isOutput=True)

    with tile.TileContext(nc) as tc:
        with tc.tile_pool(name="persist", bufs=1) as pp, \
             tc.tile_pool(name="wts", bufs=1) as wp:
            x_sb = pp.tile([P, NT, E], f32)
            yT = pp.tile([P, EC, N], bf16)
            vhat = pp.tile([P, NT, E], bf16)
            oT_all = pp.tile([P, EC, N], bf16)
            ab2_sb = pp.tile([P, H], f32)
            abf_sb = pp.tile([P, 4, NT], f32)
            inj_sb = pp.tile([1, 8, 768], f32r)
            id_sb = pp.tile([P, P], bf16)
            tri_sb = pp.tile([P, P], bf16)
            ones1 = pp.tile([1, P], f32r)
            ones64 = pp.tile([P, DH], bf16)
            magic = pp.tile([P, NT], mybir.dt.int32)
            nc.vector.memset(magic[:], 0x5F3759DF)

            x_r = x_d.rearrange("(t p) e -> p t e", p=P)
            nc.sync.dma_start(x_sb[:, 0:2, :], x_r[:, 0:2, :])
            nc.sync.dma_start(x_sb[:, 2:4, :], x_r[:, 2:4, :])
            nc.sync.dma_start(x_sb[:, 4:8, :], x_r[:, 4:8, :])
            nc.sync.dma_start(ab2_sb[:], ab2_d[:])
            nc.sync.dma_start(abf_sb[:], abf_d[:])
            nc.sync.dma_start(inj_sb[:], inj_d[:].bitcast(f32r))
            nc.sync.dma_start(id_sb[:], id_d[:])
            nc.sync.dma_start(tri_sb[:], tri_d[:])
            nc.vector.memset(ones64[:], 1.0)

            czero = pp.tile([P, 1], f32)
            cone = pp.tile([P, 1], f32)
            ceps = pp.tile([P, 1], f32)
            nc.vector.memset(czero[:], 0.0)
            nc.vector.memset(cone[:], 1.0)
            nc.vector.memset(ceps[:], EPS)
            # f32r memset hits a walrus ISA check; produce via DVE copy
            nc.vector.tensor_copy(ones1[:], cone[0:1, :].to_broadcast([1, P]))
            nc.const_aps.aps[(f32, 0.0)] = czero[:]
            nc.const_aps.aps[(f32, EPS)] = ceps[:]

            snp = pp  # stats tiles in the persistent pool via tags
            stats_in = None
            for layer in range(depth):
                _layernorm_to_yT(nc, tc, x_sb, yT, id_sb, magic, stats_in)
                _attn_layer(nc, tc, wp, layer, yT, vhat, oT_all, ab2_sb,
                            abf_sb, inj_sb, tri_sb, ones1, ones64, wqkv_d)
                st_op = snp.tile([P, NT, 2], f32, tag="stats", bufs=2, name="st_op")
                _out_proj(nc, tc, wp, layer, x_sb, oT_all, wo_d, st_op)
                _layernorm_to_yT(nc, tc, x_sb, yT, id_sb, magic, st_op)
                stats_in = snp.tile([P, NT, 2], f32, tag="stats", bufs=2, name="st_ml")
                _mlp(nc, tc, wp, layer, x_sb, yT, w1_d, b1_d, w2_d, stats_in)

            _layernorm_to_yT(nc, tc, x_sb, yT, id_sb, magic, stats_in)
            _head(nc, tc, wp, yT, oT_all, hw1_d, hb1_d, hw2_d, out_d)

    if finalize:
        nc.finalize()
    return nc


def _tile_stats(nc, pool, x_sb, t, stats):
    """per-token-tile LN stats (mean/var) -> stats[:, t, :]"""
    st6 = pool.tile([P, 2, 6], f32, tag="st6", bufs=2)
    nc.vector.bn_stats(st6[:, 0, :], x_sb[:, t, 0:384])
    nc.vector.bn_stats(st6[:, 1, :], x_sb[:, t, 384:768])
    nc.vector.bn_aggr(stats[:, t, :], st6[:])


def _ln_pools(ctx, tc):
    lp = ctx.enter_context(tc.tile_pool(name="ln", bufs=2))
    sp = ctx.enter_context(tc.tile_pool(name="lns", bufs=1))
    ps = ctx.enter_context(tc.tile_pool(name="lnp", bufs=2, space="PSUM"))
    return lp, sp, ps


def _ln_prep(sp):
    rstd = sp.tile([P, NT], f32, tag="rstd", bufs=2, name="rstd")
    nmr = sp.tile([P, NT], f32, tag="nmr", bufs=2, name="nmr")
    vpe = sp.tile([P, NT], f32, tag="vpe", bufs=2, name="vpe")
    tmp = sp.tile([P, NT], f32, tag="tmpn", bufs=2, name="tmpn")
    return rstd, nmr, vpe, tmp


def _ln_tile(nc, lp, ps, x_sb, yT, id_sb, magic, stats, ln_sc, t):
    """normalize + transpose token tile t -> yT[:, :, t*128:(t+1)*128].
    rsqrt(var+eps) via DVE bit trick + 2 Newton steps."""
    rstd, nmr, vpe, tmp = ln_sc
    i32 = mybir.dt.int32
    sl = slice(t, t + 1)
    nc.vector.tensor_scalar_add(vpe[:, sl], stats[:, sl, 1], EPS)
    nc.vector.tensor_scalar(
        rstd[:, sl].bitcast(i32), vpe[:, sl].bitcast(i32), 1, None,
        ALU.logical_shift_right,
    )
    nc.vector.tensor_tensor(
        rstd[:, sl].bitcast(i32), magic[:, sl], rstd[:, sl].bitcast(i32),
        ALU.subtract,
    )
    for _ in range(2):  # Newton: y *= 1.5 - 0.5*v*y*y
        nc.vector.tensor_tensor(tmp[:, sl], rstd[:, sl], rstd[:, sl], ALU.mult)
        nc.vector.tensor_tensor(tmp[:, sl], tmp[:, sl], vpe[:, sl], ALU.mult)
        nc.vector.tensor_scalar(tmp[:, sl], tmp[:, sl], -0.5, 1.5, ALU.mult, ALU.add)
        nc.vector.tensor_tensor(rstd[:, sl], rstd[:, sl], tmp[:, sl], ALU.mult)
    nc.vector.tensor_tensor(nmr[:, sl], stats[:, sl, 0], rstd[:, sl], ALU.mult)
    nc.vector.tensor_scalar_mul(nmr[:, sl], nmr[:, sl], -1.0)
    y_t = lp.tile([P, E], bf16, tag="y", name="y_t")
    nc.scalar.activation(
        y_t[:], x_sb[:, t, :], AF.Identity,
        bias=nmr[:, t : t + 1], scale=rstd[:, t : t + 1],
    )
    tp = ps.tile([P, E], bf16, tag="tp", name="tp")
    for c in range(EC):
        nc.tensor.transpose(tp[:, c * P : (c + 1) * P], y_t[:, c * P : (c + 1) * P], id_sb[:])
    nc.vector.tensor_copy(
        yT[:, :, t * P : (t + 1) * P],
        tp[:].rearrange("p (c q) -> p c q", q=P),
    )


def _layernorm_to_yT(nc, tc, x_sb, yT, id_sb, magic, stats=None):
    """token-major LN over x_sb -> feature-major bf16 yT (PE transpose).
    rsqrt via DVE-only bit trick + 2 Newton steps; rstd batched per half
    so normalize of tiles 0-3 doesn't wait on tile-7 stats."""
    from contextlib import ExitStack

    with ExitStack() as ctx:
        lp, sp, ps = _ln_pools(ctx, tc)
        if stats is None:
            stats = sp.tile([P, NT, 2], f32, name="stats")
            for t in range(NT):
                _tile_stats(nc, lp, x_sb, t, stats)
        rstd, nmr, vpe, tmp = _ln_prep(sp)
        i32 = mybir.dt.int32
        # rsqrt for all 8 tiles in one batch, emitted before any normalize /
        # copy work so the in-order DVE queue never blocks the transposes.
        sl = slice(0, NT)
        nc.vector.tensor_scalar_add(vpe[:, sl], stats[:, sl, 1], EPS)
        nc.vector.tensor_scalar(
            rstd[:, sl].bitcast(i32), vpe[:, sl].bitcast(i32), 1, None,
            ALU.logical_shift_right,
        )
        nc.vector.tensor_tensor(
            rstd[:, sl].bitcast(i32), magic[:, sl], rstd[:, sl].bitcast(i32),
            ALU.subtract,
        )
        for _ in range(1):  # Newton: y *= 1.5 - 0.5*v*y*y
            nc.vector.tensor_tensor(tmp[:, sl], rstd[:, sl], rstd[:, sl], ALU.mult)
            nc.vector.tensor_tensor(tmp[:, sl], tmp[:, sl], vpe[:, sl], ALU.mult)
            nc.vector.tensor_scalar(tmp[:, sl], tmp[:, sl], -0.5, 1.5, ALU.mult, ALU.add)
            nc.vector.tensor_tensor(rstd[:, sl], rstd[:, sl], tmp[:, sl], ALU.mult)
        nc.vector.tensor_tensor(nmr[:, sl], stats[:, sl, 0], rstd[:, sl], ALU.mult)
        nc.vector.tensor_scalar_mul(nmr[:, sl], nmr[:, sl], -1.0)
        if True:
            for t in range(NT):
                y_t = lp.tile([P, E], bf16, tag="y", name="y_t")
                nc.scalar.activation(
                    y_t[:], x_sb[:, t, :], AF.Identity,
                    bias=nmr[:, t : t + 1], scale=rstd[:, t : t + 1],
                )
                tp = ps.tile([P, E], bf16, tag="tp", name="tp")
                for c in range(EC):
                    nc.tensor.transpose(tp[:, c * P : (c + 1) * P], y_t[:, c * P : (c + 1) * P], id_sb[:])
                nc.vector.tensor_copy(
                    yT[:, :, t * P : (t + 1) * P],
                    tp[:].rearrange("p (c q) -> p c q", q=P),
                )


def _attn_layer(nc, tc, wp, layer, yT, vhat, oT_all, ab2_sb, abf_sb, inj_sb,
                tri_sb, ones1, ones64, wqkv_d):
    from contextlib import ExitStack

    with ExitStack() as ctx:
        qkt = ctx.enter_context(tc.tile_pool(name="qkt", bufs=1))
        psA = ctx.enter_context(tc.tile_pool(name="pqk", bufs=2, space="PSUM"))
        qp = [qkt.tile([P, N], bf16, tag=f"qp{g}", name=f"qp{g}") for g in range(EC)]
        kp = [qkt.tile([P, N], bf16, tag=f"kp{g}", name=f"kp{g}") for g in range(EC)]

        def qk_chunks(g):
            """list of 4 thunks; together they compute q,k for pair g.
            Big 512-col K=128 matmuls - woven between score groups to keep
            the PE array hot (HAM) and overlap the exp-bound stretches."""
            qw = wp.tile([P, EC, P], bf16, tag="qw", bufs=2)
            kw = wp.tile([P, EC, P], bf16, tag="kw", bufs=2)
            nc.sync.dma_start(
                qw[:], wqkv_d[layer, :, g * P : (g + 1) * P].rearrange("(o p) c -> p o c", p=P)
            )
            nc.sync.dma_start(
                kw[:], wqkv_d[layer, :, E + g * P : E + (g + 1) * P].rearrange("(o p) c -> p o c", p=P)
            )

            def one(w_sb, dst, tb):
                pq = psA.tile([P, 512], f32, tag="pqk")
                for ec in range(EC):
                    nc.tensor.matmul(
                        pq[:], w_sb[:, ec, :], yT[:, ec, tb * 512 : (tb + 1) * 512],
                        start=(ec == 0), stop=(ec == EC - 1),
                    )
                nc.vector.tensor_copy(dst[:, tb * 512 : (tb + 1) * 512], pq[:])

            return [lambda tb=tb, w=w, d=d: one(w, d, tb)
                    for tb in range(2) for (w, d) in ((qw, qp[g]), (kw, kp[g]))]

        # ---- v projection + q,k for pair 0 ----
        with tc.tile_pool(name="ppv", bufs=2, space="PSUM") as psV:
            vw = wp.tile([P, EC, E], bf16, tag="vw", bufs=1)
            nc.sync.dma_start(
                vw[:], wqkv_d[layer, :, 2 * E :].rearrange("(o p) c -> p o c", p=P)
            )
            qk0 = qk_chunks(0)
            for t in range(NT):
                pv = psV.tile([P, E], f32, tag="pv")
                for ec in range(EC):
                    nc.tensor.matmul(
                        pv[:, 0:512], yT[:, ec, t * P : (t + 1) * P], vw[:, ec, 0:512],
                        start=(ec == 0), stop=(ec == EC - 1),
                    )
                for ec in range(EC):
                    nc.tensor.matmul(
                        pv[:, 512:768], yT[:, ec, t * P : (t + 1) * P], vw[:, ec, 512:768],
                        start=(ec == 0), stop=(ec == EC - 1),
                    )
                nc.vector.tensor_copy(vhat[:, t, :], pv[:])
                if t % 2 == 1:
                    qk0[t // 2]()

        # ---- scores / exp / o / r per pair, next pair's qk woven in ----
        attp = ctx.enter_context(tc.tile_pool(name="att", bufs=3))
        afp = ctx.enter_context(tc.tile_pool(name="attf", bufs=2))
        rcp = ctx.enter_context(tc.tile_pool(name="rcp", bufs=2))
        pst = ctx.enter_context(tc.tile_pool(name="pst", bufs=2, space="PSUM"))
        pov = ctx.enter_context(tc.tile_pool(name="pov", bufs=1, space="PSUM"))
        prv = ctx.enter_context(tc.tile_pool(name="prv", bufs=1, space="PSUM"))

        for g, kind in enumerate(KIND):
            if kind == "g256":
                gen = _attn_g256(nc, g, qp[g], kp[g], vhat, oT_all, ab2_sb, inj_sb,
                                 tri_sb, ones1, ones64, attp, pst, pov, prv, rcp)
            elif kind == "g128":
                gen = _attn_g128(nc, g, qp[g], kp[g], vhat, oT_all, ab2_sb, inj_sb,
                                 tri_sb, ones1, ones64, attp, pst, pov, prv, rcp)
            else:
                gen = _attn_full(nc, g, qp[g], kp[g], vhat, oT_all, abf_sb,
                                 tri_sb, ones64, afp, pst, pov, prv, rcp)
            fill = qk_chunks(g + 1) if g + 1 < EC else []
            total = {"g256": 5, "g128": 9, "full": 10}[kind]
            nf, done = len(fill), 0
            for step in gen:
                while done < nf and (step + 1) * nf >= (done + 1) * total:
                    fill[done]()
                    done += 1
            while done < nf:
                fill[done]()
                done += 1


def _attn_g256(nc, g, qpair, kpair, vhat, oT_all, ab2_sb, inj_sb, tri_sb,
               ones1, ones64, attp, pst, pov, prv, rcp):
    """windowed heads, 256-query blocks, 3 key strips kb = 2qb-1+dd.
    attT [128, 768]: block dd holds strip kb; dd=2 only computes query cols
    [128:256] of the block. anchor A = 256qb+128; shared bias = slope*p;
    inj consts (pre-SCALE): dd=0 -2048*slope, dd=1 -1024*slope.
    o/r for block qb is emitted after scores for qb+1 (software pipeline:
    keeps the in-order PE queue from stalling on exp). Yields after each
    query-block group so the caller can weave in filler matmuls."""
    pend = None
    for qb in range(4):
        q0 = qb * 256
        ats = []
        for hh in range(2):
            h_idx = 2 * g + hh
            lo, hi = hh * DH, hh * DH + DH
            stA = pst.tile([P, 512], f32, tag=f"st{hh}")
            stB = pst.tile([P, 512], f32, tag=f"st{hh}")
            if qb >= 1:
                nc.tensor.matmul(stA[:], ones1[:], inj_sb[0:1, h_idx, 0:512],
                                 start=True, stop=False, skip_group_check=True)
                nc.tensor.matmul(
                    stA[:, 0:256], kpair[lo:hi, (2 * qb - 1) * P : 2 * qb * P],
                    qpair[lo:hi, q0 : q0 + 256],
                    start=False, stop=False, skip_group_check=True,
                )
                nc.tensor.matmul(
                    stA[:, 256:512], kpair[lo:hi, 2 * qb * P : (2 * qb + 1) * P],
                    qpair[lo:hi, q0 : q0 + 256],
                    start=False, stop=True, skip_group_check=True,
                )
            else:
                nc.tensor.matmul(stA[:, 256:512], ones1[:], inj_sb[0:1, h_idx, 256:512],
                                 start=True, stop=False, skip_group_check=True)
                nc.tensor.matmul(
                    stA[:, 256:512], kpair[lo:hi, 0:P], qpair[lo:hi, q0 : q0 + 256],
                    start=False, stop=True, skip_group_check=True,
                )
            nc.tensor.matmul(
                stB[:, 128:256], kpair[lo:hi, (2 * qb + 1) * P : (2 * qb + 2) * P],
                qpair[lo:hi, q0 + 128 : q0 + 256],
                start=True, stop=True, skip_group_check=True,
            )
            at = attp.tile([P, 768], bf16, tag=f"at{hh}")
            c0 = 256 if qb == 0 else 0
            nc.scalar.activation(
                at[:, c0:512], stA[:, c0:512], AF.Exp,
                bias=ab2_sb[:, h_idx : h_idx + 1], scale=SCALE,
            )
            nc.scalar.activation(
                at[:, 640:768], stB[:, 128:256], AF.Exp,
                bias=ab2_sb[:, h_idx : h_idx + 1], scale=SCALE,
            )
            nc.vector.tensor_tensor(at[:, 256:384], at[:, 256:384], tri_sb[:], ALU.mult)
            nc.vector.tensor_tensor(at[:, 640:768], at[:, 640:768], tri_sb[:], ALU.mult)
            ats.append(at)

        def o_r(qb=qb, q0=q0, ats=ats):
            po = pov.tile([P, 512], f32, tag="po")
            pr = prv.tile([P, 512], f32, tag="pr")
            dds = [dd for dd in range(3) if 2 * qb - 1 + dd >= 0]
            for j, dd in enumerate(dds):
                kb = 2 * qb - 1 + dd
                if dd < 2:
                    osl, asl = slice(0, 256), slice(dd * 256, dd * 256 + 256)
                else:
                    osl, asl = slice(128, 256), slice(640, 768)
                fl = dict(start=(j == 0), stop=(dd == 2), skip_group_check=True)
                for hh in range(2):
                    lo, bp = hh * DH, hh * DH
                    nc.tensor.matmul(
                        po[bp : bp + DH, osl],
                        vhat[:, kb, g * P + lo : g * P + lo + DH],
                        ats[hh][:, asl], **fl,
                    )
                for hh in range(2):
                    bp = hh * DH
                    nc.tensor.matmul(
                        pr[bp : bp + DH, osl], ones64[:], ats[hh][:, asl], **fl,
                    )
            rec = rcp.tile([P, 512], f32, tag="rec")
            nc.vector.reciprocal_approx_fast(rec[:, 0:256], pr[:, 0:256])
            nc.vector.tensor_tensor(
                oT_all[:, g, q0 : q0 + 256], po[:, 0:256], rec[:, 0:256], ALU.mult
            )

        if pend is not None:
            pend()
        pend = o_r
        yield qb
    pend()
    yield 4


def _attn_g128(nc, g, qpair, kpair, vhat, oT_all, ab2_sb, inj_sb, tri_sb,
               ones1, ones64, attp, pst, pov, prv, rcp):
    """windowed heads, 128-query blocks, strips kb = qb-1, qb.
    attT [128, 256]: [0:128] = kb=qb-1 (inj -1024*slope), [128:256] = diag.
    anchor A = 128qb+64; bias = slope*(p-64).
    o/r for block qb emitted after scores for qb+1 (software pipeline).
    Yields after each query-block group."""
    pend = None
    for qb in range(8):
        q0 = qb * P
        ats = []
        for hh in range(2):
            h_idx = 2 * g + hh
            lo, hi = hh * DH, hh * DH + DH
            st = pst.tile([P, 512], f32, tag=f"st{hh}")
            if qb >= 1:
                nc.tensor.matmul(st[:, 0:256], ones1[:], inj_sb[0:1, h_idx, 0:256],
                                 start=True, stop=False, skip_group_check=True)
                nc.tensor.matmul(
                    st[:, 0:128], kpair[lo:hi, (qb - 1) * P : qb * P],
                    qpair[lo:hi, q0 : q0 + P],
                    start=False, stop=False, skip_group_check=True,
                )
                nc.tensor.matmul(
                    st[:, 128:256], kpair[lo:hi, qb * P : (qb + 1) * P],
                    qpair[lo:hi, q0 : q0 + P],
                    start=False, stop=True, skip_group_check=True,
                )
            else:
                nc.tensor.matmul(
                    st[:, 128:256], kpair[lo:hi, 0:P], qpair[lo:hi, 0:P],
                    start=True, stop=True, skip_group_check=True,
                )
            at = attp.tile([P, 768], bf16, tag=f"at{hh}")
            c0 = 128 if qb == 0 else 0
            nc.scalar.activation(
                at[:, c0:256], st[:, c0:256], AF.Exp,
                bias=ab2_sb[:, h_idx : h_idx + 1], scale=SCALE,
            )
            nc.vector.tensor_tensor(at[:, 128:256], at[:, 128:256], tri_sb[:], ALU.mult)
            ats.append(at)

        def o_r(qb=qb, q0=q0, ats=ats):
            po = pov.tile([P, 512], f32, tag="po")
            pr = prv.tile([P, 512], f32, tag="pr")
            kbs = [qb - 1, qb] if qb >= 1 else [qb]
            for i, kb in enumerate(kbs):
                asl = slice(128 * (kb - qb + 1), 128 * (kb - qb + 2))
                fl = dict(start=(i == 0), stop=(i == len(kbs) - 1),
                          skip_group_check=True)
                for hh in range(2):
                    lo, bp = hh * DH, hh * DH
                    nc.tensor.matmul(
                        po[bp : bp + DH, 0:P],
                        vhat[:, kb, g * P + lo : g * P + lo + DH],
                        ats[hh][:, asl], **fl,
                    )
                for hh in range(2):
                    bp = hh * DH
                    nc.tensor.matmul(
                        pr[bp : bp + DH, 0:P], ones64[:], ats[hh][:, asl], **fl,
                    )
            rec = rcp.tile([P, 512], f32, tag="rec")
            nc.vector.reciprocal_approx_fast(rec[:, 0:P], pr[:, 0:P])
            nc.vector.tensor_tensor(
                oT_all[:, g, q0 : q0 + P], po[:, 0:P], rec[:, 0:P], ALU.mult
            )

        if pend is not None:
            pend()
        pend = o_r
        yield qb
    pend()
    yield 8


def _attn_full(nc, g, qpair, kpair, vhat, oT_all, abf_sb, tri_sb, ones64,
               afp, pst, pov, prv, rcp):
    """full-causal heads. strip kb covers queries [128kb, N) (W = N-128kb),
    bias = slope*(128kb+p), anchor 0. o/r per 512-query block; o/r(0) is
    emitted after strip kb=4 (software pipeline). Yields per strip."""
    at_tiles = [[None] * NT, [None] * NT]

    def o_r(qb5):
        q0 = qb5 * 512
        nkb = 4 * (qb5 + 1)
        po = pov.tile([P, 512], f32, tag="po")
        pr = prv.tile([P, 512], f32, tag="pr")
        for kb in range(nkb):
            lo_q = max(q0, kb * P)  # first query this strip covers
            w = q0 + 512 - lo_q
            osl = slice(lo_q - q0, 512)
            asl = slice(lo_q - kb * P, lo_q - kb * P + w)
            fl = dict(start=(kb == 0), stop=(kb == nkb - 1),
                      skip_group_check=True)
            for hh in range(2):
                lo, bp = hh * DH, hh * DH
                nc.tensor.matmul(
                    po[bp : bp + DH, osl],
                    vhat[:, kb, g * P + lo : g * P + lo + DH],
                    at_tiles[hh][kb][:, asl], **fl,
                )
            for hh in range(2):
                bp = hh * DH
                nc.tensor.matmul(
                    pr[bp : bp + DH, osl], ones64[:], at_tiles[hh][kb][:, asl], **fl,
                )
        rec = rcp.tile([P, 512], f32, tag="rec")
        nc.vector.reciprocal_approx_fast(rec[:], pr[:])
        nc.vector.tensor_tensor(
            oT_all[:, g, q0 : q0 + 512], po[:], rec[:], ALU.mult
        )

    for kb in range(NT):
        q0 = kb * P
        W = N - q0
        for hh in range(2):
            lo, hi = hh * DH, hh * DH + DH
            fi = (g - 4) * 2 + hh
            at = afp.tile([P, W], bf16, tag=f"af{kb}_{hh}")
            nco = 0
            while nco < W:
                w = min(512, W - nco)
                st = pst.tile([P, 512], f32, tag=f"st{hh}")
                nc.tensor.matmul(
                    st[:, 0:w],
                    kpair[lo:hi, kb * P : (kb + 1) * P],
                    qpair[lo:hi, q0 + nco : q0 + nco + w],
                    start=True, stop=True, skip_group_check=True,
                )
                nc.scalar.activation(
                    at[:, nco : nco + w], st[:, 0:w], AF.Exp,
                    bias=abf_sb[:, fi, kb : kb + 1], scale=SCALE,
                )
                nco += w
            nc.vector.tensor_tensor(at[:, 0:P], at[:, 0:P], tri_sb[:], ALU.mult)
            at_tiles[hh][kb] = at
        if kb == 4:
            o_r(0)
        yield kb
    o_r(1)
    yield 8


def _out_proj(nc, tc, wp, layer, x_sb, oT_all, wo_d, stats):
    with tc.tile_pool(name="pop", bufs=2, space="PSUM") as ps, \
         tc.tile_pool(name="ops", bufs=2) as sp:
        wo_sb = wp.tile([P, EC, E], bf16, tag="wo", bufs=1)
        nc.sync.dma_start(wo_sb[:], wo_d[layer])
        for t in range(NT):
            po = ps.tile([P, E], f32, tag="pop")
            for gc in range(EC):
                nc.tensor.matmul(
                    po[:, 0:512], oT_all[:, gc, t * P : (t + 1) * P], wo_sb[:, gc, 0:512],
                    start=(gc == 0), stop=(gc == EC - 1),
                )
            for gc in range(EC):
                nc.tensor.matmul(
                    po[:, 512:768], oT_all[:, gc, t * P : (t + 1) * P], wo_sb[:, gc, 512:768],
                    start=(gc == 0), stop=(gc == EC - 1),
                )
            nc.vector.tensor_tensor(x_sb[:, t, :], po[:], x_sb[:, t, :], ALU.add)
            _tile_stats(nc, sp, x_sb, t, stats)


def _mlp(nc, tc, wp, layer, x_sb, yT, w1_d, b1_d, w2_d, stats):
    from contextlib import ExitStack

    with ExitStack() as ctx:
        hp = ctx.enter_context(tc.tile_pool(name="hT", bufs=1))
        sp = ctx.enter_context(tc.tile_pool(name="mls", bufs=2))

        hT = hp.tile([P, FC, N], bf16)
        w2_sb = hp.tile([P, FC, E], bf16)
        b1 = wp.tile([P, FC], f32, tag="b1", bufs=2)
        nc.sync.dma_start(b1[:], b1_d[layer])
        nc.sync.dma_start(w2_sb[:], w2_d[layer].rearrange("(o p) c -> p o c", p=P))
        with tc.tile_pool(name="pf1", bufs=2, space="PSUM") as ps1:
            for fc in range(FC):
                w1c = wp.tile([P, EC, P], bf16, tag="w1c", bufs=4)
                nc.sync.dma_start(
                    w1c[:], w1_d[layer, :, fc * P : (fc + 1) * P].rearrange("(o p) c -> p o c", p=P)
                )
                pf = ps1.tile([P, N], f32, tag="pf")
                for tb in range(2):
                    for ec in range(EC):
                        nc.tensor.matmul(
                            pf[:, tb * 512 : (tb + 1) * 512], w1c[:, ec, :],
                            yT[:, ec, tb * 512 : (tb + 1) * 512],
                            start=(ec == 0), stop=(ec == EC - 1),
                        )
                nc.scalar.activation(
                    hT[:, fc, :], pf[:], AF.Gelu_apprx_tanh, bias=b1[:, fc : fc + 1]
                )
        with tc.tile_pool(name="pf2", bufs=2, space="PSUM") as ps2:
            for t in range(NT):
                pm = ps2.tile([P, E], f32, tag="pm")
                for fc in range(FC):
                    nc.tensor.matmul(
                        pm[:, 0:512], hT[:, fc, t * P : (t + 1) * P], w2_sb[:, fc, 0:512],
                        start=(fc == 0), stop=(fc == FC - 1),
                    )
                for fc in range(FC):
                    nc.tensor.matmul(
                        pm[:, 512:768], hT[:, fc, t * P : (t + 1) * P], w2_sb[:, fc, 512:768],
                        start=(fc == 0), stop=(fc == FC - 1),
                    )
                nc.vector.tensor_tensor(x_sb[:, t, :], pm[:], x_sb[:, t, :], ALU.add)
                _tile_stats(nc, sp, x_sb, t, stats)


def _head(nc, tc, wp, yT, oT_all, hw1_d, hb1_d, hw2_d, out_d):
    from contextlib import ExitStack

    h1T = oT_all  # reuse (dead after last out-proj), same shape/dtype
    with ExitStack() as ctx:
        op = ctx.enter_context(tc.tile_pool(name="ot", bufs=1))
        ps = ctx.enter_context(tc.tile_pool(name="ph", bufs=2, space="PSUM"))

        hb1 = wp.tile([P, EC], f32, tag="b1", bufs=2)
        nc.sync.dma_start(hb1[:], hb1_d[:])
        hw2 = wp.tile([P, EC, 8], bf16, tag="hw2", bufs=1)
        nc.sync.dma_start(hw2[:], hw2_d.rearrange("(o p) a -> p o a", p=P))
        for oc in range(EC):
            w1c = wp.tile([P, EC, P], bf16, tag="w1c", bufs=4)
            nc.sync.dma_start(
                w1c[:], hw1_d[:, oc * P : (oc + 1) * P].rearrange("(o p) c -> p o c", p=P)
            )
            for tb in range(2):
                pf = ps.tile([P, 512], f32, tag="pf")
                for ec in range(EC):
                    nc.tensor.matmul(
                        pf[:], w1c[:, ec, :], yT[:, ec, tb * 512 : (tb + 1) * 512],
                        start=(ec == 0), stop=(ec == EC - 1),
                    )
                nc.scalar.activation(
                    h1T[:, oc, tb * 512 : (tb + 1) * 512], pf[:], AF.Relu,
                    bias=hb1[:, oc : oc + 1],
                )
        out_sb = op.tile([P, NT, A], f32)
        for t in range(NT):
            pa = ps.tile([P, 8], f32, tag="pa")
            for ec in range(EC):
                nc.tensor.matmul(
                    pa[:], h1T[:, ec, t * P : (t + 1) * P], hw2[:, ec, :],
                    start=(ec == 0), stop=(ec == EC - 1),
                )
            nc.vector.tensor_copy(out_sb[:, t, :], pa[:, 0:A])
        nc.sync.dma_start(out_d.rearrange("(t p) a -> p t a", p=P), out_sb[:])


# ---------------------------------------------------------------- host side

_CACHE = {}


def ml_bf16():
    import ml_dtypes

    return ml_dtypes.bfloat16


def _host_prep(inputs):
    slopes = np.asarray(_slopes(H), np.float64)
    p_idx = np.arange(P, dtype=np.float64)

    ln1_s, ln1_b = np.asarray(inputs["ln1_scale"]), np.asarray(inputs["ln1_bias"])
    ln2_s, ln2_b = np.asarray(inputs["ln2_scale"]), np.asarray(inputs["ln2_bias"])
    lnf_s, lnf_b = np.asarray(inputs["lnf_scale"]), np.asarray(inputs["lnf_bias"])
    wqkv, bqkv = np.asarray(inputs["wqkv"]), np.asarray(inputs["bqkv"])
    wo, bo = np.asarray(inputs["wo"]), np.asarray(inputs["bo"])
    w1, w2 = np.asarray(inputs["w1"]), np.asarray(inputs["w2"])
    hw1, hb1 = np.asarray(inputs["head_w1"]), np.asarray(inputs["head_b1"])
    hw2 = np.asarray(inputs["head_w2"])

    # fold LN affine into following matmuls (exact algebra)
    wqkv_eff = ln1_s[:, :, None] * wqkv
    bqkv_eff = bqkv + np.einsum("le,lec->lc", ln1_b, wqkv)
    w1_eff = ln2_s[:, :, None] * w1
    b1_eff = np.einsum("le,lef->lf", ln2_b, w1)
    hw1_eff = lnf_s[:, None] * hw1
    hb1_eff = hb1 + lnf_b @ hw1

    assert np.all(bqkv_eff == 0), "nonzero qkv bias not wired"
    assert np.all(bo == 0), "nonzero out-proj bias not wired"

    bf = ml_bf16()

    # permute q/k/v columns and wo rows into pair order
    wq = wqkv_eff[:, :, 0:E].reshape(DEPTH, E, H, DH)
    wk = wqkv_eff[:, :, E : 2 * E].reshape(DEPTH, E, H, DH)
    wv = wqkv_eff[:, :, 2 * E :].reshape(DEPTH, E, H, DH)
    wqkv_p = np.concatenate(
        [wq[:, :, PERM, :].reshape(DEPTH, E, E),
         wk[:, :, PERM, :].reshape(DEPTH, E, E),
         wv[:, :, PERM, :].reshape(DEPTH, E, E)], axis=2)

    wo_r = wo.reshape(DEPTH, H, DH, E)
    wo_p = np.zeros((DEPTH, P, EC, E), np.float64)
    for gidx, (he, ho) in enumerate(PAIRS):
        wo_p[:, 0:DH, gidx, :] = wo_r[:, he]
        wo_p[:, DH:P, gidx, :] = wo_r[:, ho]

    b1_t = np.ascontiguousarray(b1_eff.reshape(DEPTH, FC, P).transpose(0, 2, 1))
    hb1_t = np.ascontiguousarray(hb1_eff.reshape(EC, P).T)

    # exp bias tables (perm order). G256: slope*p; G128: slope*(p-64);
    # full: slope*(128kb+p). inj rows are pre-SCALE (x8) units.
    ab2 = np.zeros((P, H), np.float64)
    abf = np.zeros((P, 4, NT), np.float64)
    inj = np.zeros((1, 8, 768), np.float64)
    for idx, h in enumerate(PERM):
        s = slopes[h]
        kind = KIND[idx // 2]
        if kind == "g128":
            ab2[:, idx] = s * (p_idx - 64.0)
            inj[0, idx, 0:128] = -1024.0 * s
        elif kind == "g256":
            ab2[:, idx] = s * p_idx
            inj[0, idx, 0:256] = -2048.0 * s
            inj[0, idx, 256:512] = -1024.0 * s
        else:
            fi = idx - 8
            for kb in range(NT):
                abf[:, fi, kb] = s * (128.0 * kb + p_idx)

    tri01 = (p_idx[:, None] <= p_idx[None, :]).astype(bf)  # key p <= query c

    common = {
        "wqkv_p": wqkv_p.astype(bf),
        "wo_p": wo_p.astype(bf),
        "w1_p": np.ascontiguousarray(w1_eff).astype(bf),
        "b1_t": b1_t.astype(np.float32),
        "w2_p": np.ascontiguousarray(w2).astype(bf),
        "hw1_p": np.ascontiguousarray(hw1_eff).astype(bf),
        "hb1_t": hb1_t.astype(np.float32),
        "hw2_p": np.pad(hw2, ((0, 0), (0, 1))).astype(bf),
        "ab2": ab2.astype(np.float32),
        "abf": abf.astype(np.float32),
        "inj": inj.astype(np.float32),
        "ident_bf": np.eye(P).astype(bf),
        "tri01": tri01,
    }
    return common


def kernel(**inputs):
    from concourse.bass_utils import run_bass_kernel_spmd

    common = _host_prep(inputs)
    if "nc" not in _CACHE:
        _CACHE["nc"] = build_program()
    nc = _CACHE["nc"]

    x = np.asarray(inputs["x"], np.float32)
    in_maps = [dict(common, x=np.ascontiguousarray(x[c])) for c in range(B)]
    trace = bool(int(os.environ.get("KERNEL_TRACE", "0")))
    res = run_bass_kernel_spmd(nc, in_maps, list(range(B)), trace=trace,
                               tmpdir=os.environ.get("KERNEL_TRACE_DIR"))
    if trace:
        print(f"HW exec time: {res.exec_time_ns} ns")
    return np.stack([res.results[c]["out"] for c in range(B)]).astype(np.float32)
